# revision 1
# baseline (speedup 1.0000x reference)
"""Trainium2 Bass kernel for nn_DeepCluster (vq_codebook).

Computation (per row x of shape [72]):
  7-layer MLP (Linear chain, ReLU after layers 2 and 4) -> f [200]
  sq[j]  = |f|^2 - 2*(f @ center)[j] + |center[:, j]|^2      (center: [200, 72])
  nom    = 1 / (1 + sq)                                       (alpha = 1)
  q      = nom / sum_j nom

Strategy: pure data parallel over 8 NeuronCores (batch split).  On each
core, activations flow as [features(partitions), batch(free)] bf16 tiles
of 512 rows; bf16 matmuls stream at 1 cycle/row on the PE with fast
weight load.  The distance computation runs transposed ([cluster,
batch]) so its matmuls also get a 512-wide moving operand; |c_j|^2 + 1
is added per-partition in f32 (it dominates sq, so it must not be
rounded to bf16), and the value path after the reciprocal stays f32.
PSUM->SBUF epilogues (bias add + optional ReLU) are split between the
Scalar and Vector engines.  The per-tile tail (reciprocal -> transpose
back -> row-normalize -> store) is software-pipelined one tile behind
the matmul stage so the PE never waits on the DVE round trip.
"""

import numpy as np

DIMS = [72, 128, 256, 256, 512, 512, 512, 200]
RELU_LAYERS = {2, 4}  # 1-indexed layers followed by ReLU
N_CORES = 8
N_FULL = 262144
B = 512  # rows per pipeline tile
P = 128

_CACHE = {}


def _build(n_rows):
    import concourse.bass as bass
    import concourse.mybir as mybir
    from concourse import bacc
    from concourse.tile import TileContext
    from concourse.masks import make_identity

    f32 = mybir.dt.float32
    bf16 = mybir.dt.bfloat16
    AF = mybir.ActivationFunctionType
    AX = mybir.AxisListType
    ALU = mybir.AluOpType

    kc_l = [(DIMS[i] + 127) // 128 for i in range(7)]
    mc_l = [(DIMS[i + 1] + 127) // 128 for i in range(7)]

    nc = bacc.Bacc(None, target_bir_lowering=False, debug=False)
    x_d = nc.dram_tensor("x", [n_rows, 72], bf16, kind="ExternalInput")
    q_d = nc.dram_tensor("q", [n_rows, 72], f32, kind="ExternalOutput")
    w_d, b_d = [], []
    for l in range(7):
        din, dout = DIMS[l], DIMS[l + 1]
        w_d.append(
            nc.dram_tensor(
                f"w{l + 1}", [min(din, 128), kc_l[l] * dout], bf16, kind="ExternalInput"
            )
        )
        b_d.append(nc.dram_tensor(f"b{l + 1}", [128, mc_l[l]], f32, kind="ExternalInput"))
    cm2A_d = nc.dram_tensor("cm2A", [128, 72], bf16, kind="ExternalInput")
    cm2B_d = nc.dram_tensor("cm2B", [72, 72], bf16, kind="ExternalInput")
    csq1_d = nc.dram_tensor("csq1", [72, 1], f32, kind="ExternalInput")

    n_tiles = n_rows // B
    assert n_rows % B == 0
    C = B // P  # 128-row chunks per tile

    with TileContext(nc) as tc:
        with (
            tc.tile_pool(name="consts", bufs=1) as consts,
            tc.tile_pool(name="acts", bufs=3) as acts,
            tc.tile_pool(name="pmm", bufs=4, space="PSUM") as pmm,
            tc.tile_pool(name="ptp", bufs=1, space="PSUM") as ptp,
            tc.tile_pool(name="psd", bufs=2, space="PSUM") as psd,
            tc.tile_pool(name="ppq", bufs=1, space="PSUM") as ppq,
        ):
            ones = consts.tile([128, 72], bf16, tag="ones")
            nc.vector.memset(ones, 1.0)
            ident = consts.tile([128, 128], bf16, tag="ident")
            make_identity(nc, ident)
            identf = consts.tile([128, 128], f32, tag="identf")
            make_identity(nc, identf)
            cm2A = consts.tile([128, 72], bf16, tag="cm2A")
            nc.sync.dma_start(out=cm2A, in_=cm2A_d[:])
            cm2B = consts.tile([72, 72], bf16, tag="cm2B")
            nc.sync.dma_start(out=cm2B, in_=cm2B_d[:])
            csq1 = consts.tile([72, 1], f32, tag="csq1")
            nc.sync.dma_start(out=csq1, in_=csq1_d[:])
            w_sb, b_sb = [], []
            for l in range(7):
                wt = consts.tile(list(w_d[l].shape), bf16, tag=f"w{l}")
                nc.sync.dma_start(out=wt, in_=w_d[l][:])
                w_sb.append(wt)
                bt = consts.tile([128, mc_l[l]], f32, tag=f"bias{l}")
                nc.sync.dma_start(out=bt, in_=b_d[l][:])
                b_sb.append(bt)

            x_r = x_d[:].rearrange("(t c p) j -> t p c j", p=P, c=C)
            q_r = q_d[:].rearrange("(t s p) j -> t p s j", p=P, s=C)

            def stageX(t):
                """x load + transpose -> xT [72, B] bf16 in SBUF."""
                x_sb = acts.tile([P, C, 72], bf16, tag="x")
                nc.sync.dma_start(out=x_sb, in_=x_r[t])
                ptx = ptp.tile([72, B], bf16, tag="xtp")
                for c in range(C):
                    nc.tensor.transpose(
                        ptx[:, P * c : P * (c + 1)], x_sb[:, c, :], ident
                    )
                xT = acts.tile([72, B], bf16, tag="xT")
                nc.vector.tensor_copy(xT, ptx)
                return xT

            def stageM(t, xT, next_xT_cb):
                """MLP + g + distance matmuls -> sdT PSUM.  Emits the next
                tile's input transposes mid-chain so the PE has filler work
                at layer-boundary epilogue stalls."""
                h = [xT]
                ep = 0
                for l in range(7):
                    dout = DIMS[l + 1]
                    kc, mc = kc_l[l], mc_l[l]
                    relu = (l + 1) in RELU_LAYERS
                    hn = []
                    for m in range(mc):
                        pw = min(128, dout - 128 * m)
                        ps = pmm.tile([pw, B], f32, tag="mm")
                        for k in range(kc):
                            lhsT = w_sb[l][:, k * dout + 128 * m : k * dout + 128 * m + pw]
                            nc.tensor.matmul(
                                ps, lhsT, h[k], start=(k == 0), stop=(k == kc - 1)
                            )
                        ht = acts.tile([pw, B], bf16, tag=f"h{l + 1}m{m}")
                        bias_col = b_sb[l][:pw, m : m + 1]
                        if ep % 2 == 0:  # scalar engine (ACT)
                            nc.scalar.activation(
                                out=ht,
                                in_=ps,
                                func=AF.Relu if relu else AF.Identity,
                                bias=bias_col,
                                scale=1.0,
                            )
                        else:  # vector engine (DVE)
                            if relu:
                                nc.vector.tensor_scalar(
                                    out=ht,
                                    in0=ps,
                                    scalar1=bias_col,
                                    scalar2=0.0,
                                    op0=ALU.add,
                                    op1=ALU.max,
                                )
                            else:
                                nc.vector.tensor_scalar_add(ht, ps, bias_col)
                        ep += 1
                        hn.append(ht)
                    h = hn
                    if l == 1 and next_xT_cb is not None:
                        next_xT_cb()

                f0, f1 = h  # [128, B], [72, B] bf16
                g0 = acts.tile([128, B], bf16, tag="g0")
                nc.vector.tensor_mul(g0, f0, f0)
                g1 = acts.tile([72, B], bf16, tag="g1")
                nc.vector.tensor_mul(g1, f1, f1)

                sdT = psd.tile([72, B], f32, tag="sd")
                nc.tensor.matmul(sdT, ones[:128, :72], g0, start=True, stop=False)
                nc.tensor.matmul(sdT, ones[:72, :72], g1, start=False, stop=False)
                nc.tensor.matmul(sdT, cm2A, f0, start=False, stop=False)
                nc.tensor.matmul(sdT, cm2B, f1, start=False, stop=True)
                return sdT

            def stageB(t, sdT):
                """csq add + reciprocal + transpose back + normalize + store."""
                sd1 = acts.tile([72, B], f32, tag="sd1")
                nc.scalar.activation(
                    out=sd1, in_=sdT, func=AF.Identity, bias=csq1[:, 0:1], scale=1.0
                )
                nomT = acts.tile([72, B], f32, tag="nomT")
                nc.vector.reciprocal_approx_fast(out=nomT, in_=sd1)

                pq = ppq.tile([P, C, 72], f32, tag="pq")
                for s in range(C):
                    nc.tensor.transpose(
                        pq[:, s, :], nomT[:, P * s : P * (s + 1)], identf[:72, :72]
                    )
                rs4 = acts.tile([P, C], f32, tag="rs4")
                nc.vector.reduce_sum(rs4, pq, axis=AX.X)
                rr4 = acts.tile([P, C], f32, tag="rr4")
                nc.vector.reciprocal(rr4, rs4)
                rr_b = bass.AP(
                    tensor=rr4.tensor,
                    offset=rr4.offset,
                    ap=[rr4.ap[0], rr4.ap[1], [0, 72]],
                )
                qt = acts.tile([P, C, 72], f32, tag="qt")
                nc.vector.tensor_tensor(out=qt, in0=pq, in1=rr_b, op=ALU.mult)
                nc.sync.dma_start(out=q_r[t], in_=qt)

            prev = None
            next_xT = [stageX(0)]

            for t in range(n_tiles):

                def make_cb(tn):
                    if tn >= n_tiles:
                        return None

                    def cb():
                        next_xT.append(stageX(tn))

                    return cb

                cur = (t, stageM(t, next_xT.pop(0), make_cb(t + 1)))
                if prev is not None:
                    stageB(*prev)
                prev = cur
            stageB(*prev)

    nc.compile()
    return nc


def _prep_consts(ws, bs, center):
    """Host-side marshalling of the small replicated weights."""
    import ml_dtypes

    bf = ml_dtypes.bfloat16
    kc_l = [(DIMS[i] + 127) // 128 for i in range(7)]
    mc_l = [(DIMS[i + 1] + 127) // 128 for i in range(7)]
    consts = {}
    for l in range(7):
        din, dout = DIMS[l], DIMS[l + 1]
        w = np.ascontiguousarray(ws[l], dtype=np.float32)
        if din > 128:
            kc = kc_l[l]
            w = np.ascontiguousarray(
                w.reshape(kc, 128, dout).transpose(1, 0, 2).reshape(128, kc * dout)
            )
        consts[f"w{l + 1}"] = w.astype(bf)
        bt = np.zeros((128, mc_l[l]), dtype=np.float32)
        for m in range(mc_l[l]):
            pw = min(128, dout - 128 * m)
            bt[:pw, m] = bs[l][128 * m : 128 * m + pw]
        consts[f"b{l + 1}"] = bt
    c = np.asarray(center, dtype=np.float32)
    consts["cm2A"] = np.ascontiguousarray(-2.0 * c[:128, :]).astype(bf)
    consts["cm2B"] = np.ascontiguousarray(-2.0 * c[128:, :]).astype(bf)
    consts["csq1"] = np.ascontiguousarray(
        (1.0 + (c.astype(np.float64) ** 2).sum(axis=0)).reshape(72, 1)
    ).astype(np.float32)
    return consts


def kernel(
    inputs, w1, b1, w2, b2, w3, b3, w4, b4, w5, b5, w6, b6, w7, b7, center
):
    import ml_dtypes
    from concourse.bass_utils import run_bass_kernel_spmd

    x = np.asarray(inputs).astype(ml_dtypes.bfloat16)
    n = x.shape[0]
    n_loc = n // N_CORES
    key = n_loc
    if key not in _CACHE:
        _CACHE[key] = _build(n_loc)
    nc = _CACHE[key]

    consts = _prep_consts(
        [w1, w2, w3, w4, w5, w6, w7], [b1, b2, b3, b4, b5, b6, b7], center
    )
    in_maps = []
    for c in range(N_CORES):
        m = {"x": np.ascontiguousarray(x[c * n_loc : (c + 1) * n_loc])}
        m.update(consts)
        in_maps.append(m)
    res = run_bass_kernel_spmd(nc, in_maps, core_ids=list(range(N_CORES)))
    return np.concatenate([res.results[c]["q"] for c in range(N_CORES)], axis=0)



# revision 4
# speedup vs baseline: 3.0776x; 3.0776x over previous
"""Trainium2 Bass kernel for nn_DeepCluster (vq_codebook).

Math (per row x in R^72):
  7-layer MLP, ReLU only after layers 2 and 4  ->  f in R^200
  sq[j] = |f - center[:, j]|^2 ;  q = (1/(1+sq)) / sum_j (1/(1+sq))

Because ReLU appears only after layers 2 and 4, the layer chains 1-2,
3-4 and 5-6-7 are affine compositions.  The host pre-multiplies them in
float64 into three matrices W12 [72,256], W34 [256,512], W567 [512,200]
(3.25x fewer matmul FLOPs than the literal 7-layer chain).  The device
then computes, per 512-row tile (feature-major layout [feat, batch]):

  A: h2 = relu(x @ W12 + b12)        2 bf16 matmuls (bias via ones-row)
  B: h4 = relu(h2 @ W34 + b34)       4 fp8 DoubleRow matmuls (K=256)
  C: e  = h4 @ W567                  4 fp8 DoubleRow matmuls (K=512)
  D: sq = |e|^2 - 2 e.(c-b567) ...   2 fp8 DoubleRow matmuls (K=200)
  tail: 1/(1+sq), transpose back, row-normalize, store

The C-layer bias b567 is folded into shifted centers c' = c - b567 and
|c'|^2, so B is the only stage needing per-feature bias epilogues.
Activations are scaled by per-stage powers of two (calibrated on a host
subsample) to sit in fp8e4 range; all scale folds are exact.
Work is software-pipelined 3 tiles deep and spread over ACT/DVE/GPSIMD
so the PE (12 matmuls + 4 transposes per tile) stays the critical path.
"""

import numpy as np

N_CORES = 8
B = 512  # rows per pipeline tile
P = 128

_CACHE = {}


def _build(n_rows, cA, sC, sD):
    import concourse.bass as bass
    import concourse.mybir as mybir
    from concourse import bacc
    from concourse.tile import TileContext
    from concourse.masks import make_identity

    f32 = mybir.dt.float32
    bf16 = mybir.dt.bfloat16
    fp8 = mybir.dt.float8e4
    AF = mybir.ActivationFunctionType
    AX = mybir.AxisListType
    ALU = mybir.AluOpType
    DR = mybir.MatmulPerfMode.DoubleRow

    nc = bacc.Bacc(None, target_bir_lowering=False, debug=False)
    xt_d = nc.dram_tensor("xt", [73, n_rows], bf16, kind="ExternalInput")
    q_d = nc.dram_tensor("q", [n_rows, 72], f32, kind="ExternalOutput")
    w12_d = nc.dram_tensor("w12", [73, 256], bf16, kind="ExternalInput")
    w34_d = nc.dram_tensor("w34", [128, 1024], fp8, kind="ExternalInput")
    w567_d = nc.dram_tensor("w567", [128, 896], fp8, kind="ExternalInput")
    cm2_d = nc.dram_tensor("cm2", [100, 160], fp8, kind="ExternalInput")
    onesd_d = nc.dram_tensor("onesd", [100, 160], fp8, kind="ExternalInput")
    bb_d = nc.dram_tensor("bb", [128, 4], f32, kind="ExternalInput")
    csq_d = nc.dram_tensor("csq", [72, 1], f32, kind="ExternalInput")

    n_tiles = n_rows // B
    assert n_rows % B == 0
    C = B // P

    with TileContext(nc) as tc:
        with (
            tc.tile_pool(name="consts", bufs=1) as consts,
            tc.tile_pool(name="xt", bufs=3) as xtp,
            tc.tile_pool(name="acts", bufs=2) as acts,
            tc.tile_pool(name="fg", bufs=3) as fgp,
            tc.tile_pool(name="pmm", bufs=3, space="PSUM") as pmm,
            tc.tile_pool(name="pd", bufs=1, space="PSUM") as pdp,
            tc.tile_pool(name="pt", bufs=1, space="PSUM") as ptp,
        ):
            identf = consts.tile([128, 128], f32, tag="identf")
            make_identity(nc, identf)
            w12 = consts.tile([73, 256], bf16, tag="w12")
            nc.sync.dma_start(out=w12, in_=w12_d[:])
            w34 = consts.tile([128, 2, 512], fp8, tag="w34")
            nc.sync.dma_start(out=w34, in_=w34_d[:].rearrange("p (i m) -> p i m", i=2))
            w567 = consts.tile([128, 4, 224], fp8, tag="w567")
            nc.sync.dma_start(
                out=w567, in_=w567_d[:].rearrange("p (i m) -> p i m", i=4)
            )
            cm2 = consts.tile([100, 2, 80], fp8, tag="cm2")
            nc.sync.dma_start(out=cm2, in_=cm2_d[:].rearrange("p (i m) -> p i m", i=2))
            onesd = consts.tile([100, 2, 80], fp8, tag="onesd")
            nc.sync.dma_start(
                out=onesd, in_=onesd_d[:].rearrange("p (i m) -> p i m", i=2)
            )
            bb = consts.tile([128, 4], f32, tag="bb")
            nc.sync.dma_start(out=bb, in_=bb_d[:])
            csq = consts.tile([72, 1], f32, tag="csq")
            nc.sync.dma_start(out=csq, in_=csq_d[:])

            q_r = q_d[:].rearrange("(t s p) j -> t p s j", p=P, s=C)

            xt_sb = [None] * n_tiles
            h2_sb = [None] * n_tiles
            h4_sb = [None] * n_tiles
            f_sb = [None] * n_tiles
            g_sb = [None] * n_tiles
            sd_sb = [None] * n_tiles
            nom_sb = [None] * n_tiles

            def load(t):
                xt_sb[t] = xtp.tile([73, B], bf16, name="xt", tag="x")
                nc.sync.dma_start(out=xt_sb[t], in_=xt_d[:, B * t : B * (t + 1)])

            def stageA(t):
                ps = pmm.tile([128, 2, B], f32, name="psmm", tag="mm")
                for m in range(2):
                    nc.tensor.matmul(
                        ps[:, m, :], w12[:, 128 * m : 128 * (m + 1)], xt_sb[t],
                        start=True, stop=True,
                    )
                h2_sb[t] = acts.tile([128, 2, B], fp8, name="h2", tag="h2")
                nc.scalar.activation(
                    out=h2_sb[t], in_=ps, func=AF.Relu, bias=0.0, scale=cA
                )
                xt_sb[t] = None

            def stageB(t):
                pss = []
                for half in range(2):
                    ps = pmm.tile([128, 2, B], f32, name="psmm", tag="mm")
                    for mi in range(2):
                        m = 2 * half + mi
                        nc.tensor.matmul(
                            ps[:, mi, :],
                            w34[:, :, 128 * m : 128 * (m + 1)],
                            h2_sb[t],
                            start=True, stop=True, perf_mode=DR,
                        )
                    pss.append(ps)
                h4_sb[t] = acts.tile([128, 4, B], fp8, name="h4", tag="h4")
                for m in range(4):
                    ps = pss[m // 2][:, m % 2, :]
                    dst = h4_sb[t][:, m, :]
                    bias_col = bb[:, m : m + 1]
                    if m < 2:
                        nc.scalar.activation(
                            out=dst, in_=ps, func=AF.Relu, bias=bias_col, scale=1.0
                        )
                    else:
                        nc.vector.tensor_scalar(
                            out=dst, in0=ps, scalar1=bias_col, scalar2=0.0,
                            op0=ALU.add, op1=ALU.max,
                        )
                h2_sb[t] = None

            def stageC(t):
                ps = pmm.tile([100, 2, B], f32, name="psc", tag="mm")
                for mp in range(2):
                    for c in range(2):
                        nc.tensor.matmul(
                            ps[:, mp, :],
                            w567[:, 2 * c : 2 * c + 2, 112 * mp : 112 * mp + 100],
                            h4_sb[t][:, 2 * c : 2 * c + 2, :],
                            start=(c == 0), stop=(c == 1), perf_mode=DR,
                        )
                f_sb[t] = fgp.tile([100, 2, B], fp8, name="ft", tag="f")
                nc.vector.tensor_scalar_mul(f_sb[t], ps, sC)
                g_sb[t] = fgp.tile([100, 2, B], fp8, name="gt", tag="g")
                nc.gpsimd.tensor_mul(g_sb[t], f_sb[t], f_sb[t])
                h4_sb[t] = None

            def stageD(t):
                ps = pdp.tile([72, B], f32, name="psd", tag="sd")
                nc.tensor.matmul(
                    ps, cm2[:, :, 0:72], f_sb[t], start=True, stop=False, perf_mode=DR
                )
                nc.tensor.matmul(
                    ps, onesd[:, :, 0:72], g_sb[t], start=False, stop=True, perf_mode=DR
                )
                sd_sb[t] = acts.tile([72, B], f32, name="sd", tag="sdp")
                nc.scalar.activation(
                    out=sd_sb[t], in_=ps, func=AF.Identity, bias=csq[:, 0:1], scale=sD
                )
                f_sb[t] = None
                g_sb[t] = None

            def stageT(t):
                pq = ptp.tile([P, C, 72], f32, name="pq", tag="pq")
                for s in range(C):
                    nc.tensor.transpose(
                        pq[:, s, :], sd_sb[t][:, P * s : P * (s + 1)], identf[:72, :72]
                    )
                sd_sb[t] = None
                nom_sb[t] = acts.tile([P, C, 72], f32, name="nom", tag="nom")
                nc.vector.reciprocal_approx_fast(out=nom_sb[t], in_=pq)

            def tail(t):
                nom = nom_sb[t]
                rs = acts.tile([P, C], f32, name="rs", tag="rs")
                nc.vector.reduce_sum(rs, nom, axis=AX.X)
                rr = acts.tile([P, C], f32, name="rr", tag="rr")
                nc.vector.reciprocal(rr, rs)
                rr_b = bass.AP(
                    tensor=rr.tensor,
                    offset=rr.offset,
                    ap=[rr.ap[0], rr.ap[1], [0, 72]],
                )
                qt = acts.tile([P, C, 72], f32, name="qt", tag="qt")
                nc.gpsimd.tensor_tensor(out=qt, in0=nom, in1=rr_b, op=ALU.mult)
                nc.sync.dma_start(out=q_r[t], in_=qt)
                nom_sb[t] = None

            load(0)
            load(1)
            stageA(0)
            for t in range(n_tiles + 3):
                if t + 2 < n_tiles:
                    load(t + 2)
                if 0 <= t - 3:
                    stageT(t - 3)
                if 0 <= t - 2 < n_tiles:
                    stageD(t - 2)
                if 0 <= t - 3:
                    tail(t - 3)
                if t < n_tiles:
                    stageB(t)
                if t + 1 < n_tiles:
                    stageA(t + 1)
                if t < n_tiles:
                    stageC(t)

    nc.compile()
    return nc


def _pow2(v):
    return float(2.0 ** np.round(np.log2(v)))


def prepare(inputs_np):
    """Host-side marshalling: merge affine chains in f64, calibrate fp8
    scales on a subsample, quantize, build per-core input maps."""
    import ml_dtypes

    bf = ml_dtypes.bfloat16
    f8 = ml_dtypes.float8_e4m3

    x = np.asarray(inputs_np["inputs"], dtype=np.float64)
    ws = [np.asarray(inputs_np[f"w{i}"], dtype=np.float64) for i in range(1, 8)]
    bs = [np.asarray(inputs_np[f"b{i}"], dtype=np.float64) for i in range(1, 8)]
    center = np.asarray(inputs_np["center"], dtype=np.float64)

    W12 = ws[0] @ ws[1]
    b12 = bs[0] @ ws[1] + bs[1]
    W34 = ws[2] @ ws[3]
    b34 = bs[2] @ ws[3] + bs[3]
    W567 = ws[4] @ ws[5] @ ws[6]
    b567 = (bs[4] @ ws[5] + bs[5]) @ ws[6] + bs[6]
    cp = center - b567[:, None]  # shifted centers c' = c - b567, [200, 72]

    n = x.shape[0]
    sub = x[:: max(1, n // 4096)][:4096]
    h2 = np.maximum(sub @ W12 + b12, 0.0)
    h4 = np.maximum(h2 @ W34 + b34, 0.0)
    e = h4 @ W567

    def rms(a):
        return float(np.sqrt(np.mean(a.astype(np.float64) ** 2)) + 1e-30)

    cA = _pow2(1.0 / rms(h2))
    kB = _pow2(0.25 / rms(W34))
    # keep the (kB*cA)-scaled h4 inside fp8 range
    while kB * cA * rms(h4) > 8.0:
        kB /= 2.0
    kC = _pow2(0.25 / rms(W567))
    cF = min(_pow2(1.0 / rms(e)), 256.0)
    sC = cF / (kC * kB * cA)
    sD = 1.0 / cF  # kD == cF

    def q8(a):
        return np.clip(a, -224.0, 224.0).astype(f8)

    consts = {}
    consts["w12"] = np.concatenate([W12, b12[None, :]], axis=0).astype(bf)
    w34t = np.zeros((128, 2, 512), dtype=np.float64)
    for i in range(2):
        w34t[:, i, :] = kB * W34[128 * i : 128 * (i + 1), :]
    consts["w34"] = q8(w34t.reshape(128, 1024))
    w567t = np.zeros((128, 4, 224), dtype=np.float64)
    for c in range(2):
        for i in range(2):
            blk = kC * W567[256 * c + 128 * i : 256 * c + 128 * (i + 1), :]
            w567t[:, 2 * c + i, 0:100] = blk[:, 0:100]
            w567t[:, 2 * c + i, 112:212] = blk[:, 100:200]
    consts["w567"] = q8(w567t.reshape(128, 896))
    cm2t = np.zeros((100, 2, 80), dtype=np.float64)
    onest = np.zeros((100, 2, 80), dtype=np.float64)
    for i in range(2):
        cm2t[:, i, 0:72] = -2.0 * cp[100 * i : 100 * (i + 1), :]
        onest[:, i, 0:72] = 1.0 / cF
    consts["cm2"] = q8(cm2t.reshape(100, 160))
    consts["onesd"] = q8(onest.reshape(100, 160))
    bbt = np.zeros((128, 4), dtype=np.float64)
    for m in range(4):
        bbt[:, m] = kB * cA * b34[128 * m : 128 * (m + 1)]
    consts["bb"] = bbt.astype(np.float32)
    consts["csq"] = (1.0 + (cp**2).sum(axis=0)).reshape(72, 1).astype(np.float32)

    n_loc = n // N_CORES
    key = (n_loc, cA, sC, sD)
    if key not in _CACHE:
        _CACHE[key] = _build(n_loc, cA, sC, sD)
    nc = _CACHE[key]

    in_maps = []
    for c in range(N_CORES):
        xt = np.empty((73, n_loc), dtype=bf)
        xt[:72] = x[c * n_loc : (c + 1) * n_loc].T
        xt[72] = 1.0
        m = {"xt": np.ascontiguousarray(xt)}
        m.update(consts)
        in_maps.append(m)
    return nc, in_maps


def kernel(
    inputs, w1, b1, w2, b2, w3, b3, w4, b4, w5, b5, w6, b6, w7, b7, center
):
    from concourse.bass_utils import run_bass_kernel_spmd

    inputs_np = {
        "inputs": inputs, "center": center,
        "w1": w1, "b1": b1, "w2": w2, "b2": b2, "w3": w3, "b3": b3,
        "w4": w4, "b4": b4, "w5": w5, "b5": b5, "w6": w6, "b6": b6,
        "w7": w7, "b7": b7,
    }
    nc, in_maps = prepare(inputs_np)
    res = run_bass_kernel_spmd(nc, in_maps, core_ids=list(range(N_CORES)))
    return np.concatenate([res.results[c]["q"] for c in range(N_CORES)], axis=0)


# revision 8
# speedup vs baseline: 3.1617x; 1.0273x over previous
"""Trainium2 Bass kernel for nn_DeepCluster (vq_codebook).

Math (per row x in R^72):
  7-layer MLP, ReLU only after layers 2 and 4  ->  f in R^200
  sq[j] = |f - center[:, j]|^2 ;  q = (1/(1+sq)) / sum_j (1/(1+sq))

Because ReLU appears only after layers 2 and 4, the layer chains 1-2,
3-4 and 5-6-7 are affine compositions.  The host pre-multiplies them in
float64 into three matrices W12 [72,256], W34 [256,512], W567 [512,200]
(3.25x fewer matmul FLOPs than the literal 7-layer chain).  The device
then computes, per 512-row tile (feature-major layout [feat, batch]):

  A: h2 = relu(x @ W12 + b12)        2 bf16 matmuls (bias via ones-row)
  B: h4 = relu(h2 @ W34 + b34)       4 fp8 DoubleRow matmuls (K=256)
  C: e  = h4 @ W567                  4 fp8 DoubleRow matmuls (K=512)
  D: sq = |e|^2 - 2 e.(c-b567) ...   2 fp8 DoubleRow matmuls (K=200)
  tail: 1/(1+sq), transpose back, row-normalize, store

The C-layer bias b567 is folded into shifted centers c' = c - b567 and
|c'|^2, so B is the only stage needing per-feature bias epilogues.
Activations are scaled by per-stage powers of two (calibrated on a host
subsample) to sit in fp8e4 range; all scale folds are exact.
Work is software-pipelined 3 tiles deep and spread over ACT/DVE/GPSIMD
so the PE (12 matmuls + 4 transposes per tile) stays the critical path.
"""

import numpy as np

N_CORES = 8
B = 512  # rows per pipeline tile
P = 128

_CACHE = {}


def _build(n_rows, cA, sC, sD):
    import concourse.bass as bass
    import concourse.mybir as mybir
    from concourse import bacc
    from concourse.tile import TileContext
    from concourse.masks import make_identity

    f32 = mybir.dt.float32
    bf16 = mybir.dt.bfloat16
    fp8 = mybir.dt.float8e4
    AF = mybir.ActivationFunctionType
    AX = mybir.AxisListType
    ALU = mybir.AluOpType
    DR = mybir.MatmulPerfMode.DoubleRow

    nc = bacc.Bacc(None, target_bir_lowering=False, debug=False)
    xt_d = nc.dram_tensor("xt", [73, n_rows], bf16, kind="ExternalInput")
    q_d = nc.dram_tensor("q", [n_rows, 72], f32, kind="ExternalOutput")
    w12_d = nc.dram_tensor("w12", [73, 256], bf16, kind="ExternalInput")
    w34_d = nc.dram_tensor("w34", [128, 1024], fp8, kind="ExternalInput")
    w567_d = nc.dram_tensor("w567", [128, 896], fp8, kind="ExternalInput")
    cm2_d = nc.dram_tensor("cm2", [100, 160], fp8, kind="ExternalInput")
    onesd_d = nc.dram_tensor("onesd", [100, 160], fp8, kind="ExternalInput")
    bb_d = nc.dram_tensor("bb", [128, 4], f32, kind="ExternalInput")
    csq_d = nc.dram_tensor("csq", [72, 1], f32, kind="ExternalInput")

    n_tiles = n_rows // B
    assert n_rows % B == 0
    C = B // P

    with TileContext(nc) as tc:
        with (
            tc.tile_pool(name="consts", bufs=1) as consts,
            tc.tile_pool(name="xt", bufs=4) as xtp,
            tc.tile_pool(name="acts", bufs=2) as acts,
            tc.tile_pool(name="h2p", bufs=3) as h2p,
            tc.tile_pool(name="fg", bufs=3) as fgp,
            tc.tile_pool(name="pmm", bufs=3, space="PSUM") as pmm,
            tc.tile_pool(name="pd", bufs=1, space="PSUM") as pdp,
            tc.tile_pool(name="pt", bufs=1, space="PSUM") as ptp,
        ):
            identf = consts.tile([128, 128], f32, tag="identf")
            make_identity(nc, identf)
            w12 = consts.tile([73, 256], bf16, tag="w12")
            nc.sync.dma_start(out=w12, in_=w12_d[:])
            w34 = consts.tile([128, 2, 512], fp8, tag="w34")
            nc.sync.dma_start(out=w34, in_=w34_d[:].rearrange("p (i m) -> p i m", i=2))
            w567 = consts.tile([128, 4, 224], fp8, tag="w567")
            nc.sync.dma_start(
                out=w567, in_=w567_d[:].rearrange("p (i m) -> p i m", i=4)
            )
            cm2 = consts.tile([100, 2, 80], fp8, tag="cm2")
            nc.sync.dma_start(out=cm2, in_=cm2_d[:].rearrange("p (i m) -> p i m", i=2))
            onesd = consts.tile([100, 2, 80], fp8, tag="onesd")
            nc.sync.dma_start(
                out=onesd, in_=onesd_d[:].rearrange("p (i m) -> p i m", i=2)
            )
            bb = consts.tile([128, 4], f32, tag="bb")
            nc.sync.dma_start(out=bb, in_=bb_d[:])
            csq = consts.tile([72, 1], f32, tag="csq")
            nc.sync.dma_start(out=csq, in_=csq_d[:])

            q_r = q_d[:].rearrange("(t s p) j -> t p s j", p=P, s=C)

            xt_sb = [None] * n_tiles
            h2_sb = [None] * n_tiles
            h4_sb = [None] * n_tiles
            f_sb = [None] * n_tiles
            g_sb = [None] * n_tiles
            sd_sb = [None] * n_tiles
            nom_sb = [None] * n_tiles

            def load(t):
                xt_sb[t] = xtp.tile([73, B], bf16, name="xt", tag="x")
                nc.sync.dma_start(out=xt_sb[t], in_=xt_d[:, B * t : B * (t + 1)])

            def stageA(t):
                ps = pmm.tile([128, 2, B], f32, name="psmm", tag="mm")
                for m in range(2):
                    nc.tensor.matmul(
                        ps[:, m, :], w12[:, 128 * m : 128 * (m + 1)], xt_sb[t],
                        start=True, stop=True,
                    )
                h2_sb[t] = h2p.tile([128, 2, B], fp8, name="h2", tag="h2")
                nc.scalar.activation(
                    out=h2_sb[t], in_=ps, func=AF.Relu, bias=0.0, scale=cA
                )
                xt_sb[t] = None

            def stageB(t):
                pss = []
                for half in range(2):
                    ps = pmm.tile([128, 2, B], f32, name="psmm", tag="mm")
                    for mi in range(2):
                        m = 2 * half + mi
                        nc.tensor.matmul(
                            ps[:, mi, :],
                            w34[:, :, 128 * m : 128 * (m + 1)],
                            h2_sb[t],
                            start=True, stop=True, perf_mode=DR,
                        )
                    pss.append(ps)
                h4_sb[t] = acts.tile([128, 4, B], fp8, name="h4", tag="h4")
                for m in range(4):
                    ps = pss[m // 2][:, m % 2, :]
                    dst = h4_sb[t][:, m, :]
                    bias_col = bb[:, m : m + 1]
                    if m < 2:
                        nc.scalar.activation(
                            out=dst, in_=ps, func=AF.Relu, bias=bias_col, scale=1.0
                        )
                    else:
                        nc.vector.tensor_scalar(
                            out=dst, in0=ps, scalar1=bias_col, scalar2=0.0,
                            op0=ALU.add, op1=ALU.max,
                        )
                h2_sb[t] = None

            def stageC(t):
                ps = pmm.tile([100, 2, B], f32, name="psc", tag="mm")
                for c in range(2):
                    for mp in range(2):
                        nc.tensor.matmul(
                            ps[:, mp, :],
                            w567[:, 2 * c : 2 * c + 2, 112 * mp : 112 * mp + 100],
                            h4_sb[t][:, 2 * c : 2 * c + 2, :],
                            start=(c == 0), stop=(c == 1), perf_mode=DR,
                        )
                f_sb[t] = fgp.tile([100, 2, B], fp8, name="ft", tag="f")
                nc.vector.tensor_scalar_mul(f_sb[t], ps, sC)
                g_sb[t] = fgp.tile([100, 2, B], fp8, name="gt", tag="g")
                nc.gpsimd.tensor_mul(g_sb[t], f_sb[t], f_sb[t])
                h4_sb[t] = None

            def stageD(t):
                ps = pdp.tile([72, B], f32, name="psd", tag="sd")
                nc.tensor.matmul(
                    ps, cm2[:, :, 0:72], f_sb[t], start=True, stop=False, perf_mode=DR
                )
                nc.tensor.matmul(
                    ps, onesd[:, :, 0:72], g_sb[t], start=False, stop=True, perf_mode=DR
                )
                sd_sb[t] = acts.tile([72, B], f32, name="sd", tag="sdp")
                nc.scalar.activation(
                    out=sd_sb[t], in_=ps, func=AF.Identity, bias=csq[:, 0:1], scale=sD
                )
                f_sb[t] = None
                g_sb[t] = None

            def stageT(t):
                pq = ptp.tile([P, C, 72], f32, name="pq", tag="pq")
                for s in range(C):
                    nc.tensor.transpose(
                        pq[:, s, :], sd_sb[t][:, P * s : P * (s + 1)], identf[:72, :72]
                    )
                sd_sb[t] = None
                nom_sb[t] = acts.tile([P, C, 72], f32, name="nom", tag="nom")
                nc.vector.reciprocal_approx_fast(out=nom_sb[t], in_=pq)

            def tail(t):
                nom = nom_sb[t]
                rs = acts.tile([P, C], f32, name="rs", tag="rs")
                nc.vector.reduce_sum(rs, nom, axis=AX.X)
                rr = acts.tile([P, C], f32, name="rr", tag="rr")
                nc.vector.reciprocal(rr, rs)
                rr_b = bass.AP(
                    tensor=rr.tensor,
                    offset=rr.offset,
                    ap=[rr.ap[0], rr.ap[1], [0, 72]],
                )
                qt = acts.tile([P, C, 72], f32, name="qt", tag="qt")
                nc.gpsimd.tensor_tensor(out=qt, in0=nom, in1=rr_b, op=ALU.mult)
                nc.sync.dma_start(out=q_r[t], in_=qt)
                nom_sb[t] = None

            load(0)
            load(1)
            stageA(0)
            load(2)
            stageA(1)
            for t in range(n_tiles + 3):
                if t + 3 < n_tiles:
                    load(t + 3)
                if 0 <= t - 3:
                    stageT(t - 3)
                if 0 <= t - 2 < n_tiles:
                    stageD(t - 2)
                if 0 <= t - 3:
                    tail(t - 3)
                if t < n_tiles:
                    stageB(t)
                if t + 2 < n_tiles:
                    stageA(t + 2)
                if t < n_tiles:
                    stageC(t)

    nc.compile()
    return nc


def _pow2(v):
    return float(2.0 ** np.round(np.log2(v)))


def prepare(inputs_np):
    """Host-side marshalling: merge affine chains in f64, calibrate fp8
    scales on a subsample, quantize, build per-core input maps."""
    import ml_dtypes

    bf = ml_dtypes.bfloat16
    f8 = ml_dtypes.float8_e4m3

    x = np.asarray(inputs_np["inputs"], dtype=np.float64)
    ws = [np.asarray(inputs_np[f"w{i}"], dtype=np.float64) for i in range(1, 8)]
    bs = [np.asarray(inputs_np[f"b{i}"], dtype=np.float64) for i in range(1, 8)]
    center = np.asarray(inputs_np["center"], dtype=np.float64)

    W12 = ws[0] @ ws[1]
    b12 = bs[0] @ ws[1] + bs[1]
    W34 = ws[2] @ ws[3]
    b34 = bs[2] @ ws[3] + bs[3]
    W567 = ws[4] @ ws[5] @ ws[6]
    b567 = (bs[4] @ ws[5] + bs[5]) @ ws[6] + bs[6]
    cp = center - b567[:, None]  # shifted centers c' = c - b567, [200, 72]

    n = x.shape[0]
    sub = x[:: max(1, n // 4096)][:4096]
    h2 = np.maximum(sub @ W12 + b12, 0.0)
    h4 = np.maximum(h2 @ W34 + b34, 0.0)
    e = h4 @ W567

    def rms(a):
        return float(np.sqrt(np.mean(a.astype(np.float64) ** 2)) + 1e-30)

    cA = _pow2(1.0 / rms(h2))
    kB = _pow2(0.25 / rms(W34))
    # keep the (kB*cA)-scaled h4 inside fp8 range
    while kB * cA * rms(h4) > 8.0:
        kB /= 2.0
    kC = _pow2(0.25 / rms(W567))
    cF = min(_pow2(1.0 / rms(e)), 256.0)
    sC = cF / (kC * kB * cA)
    sD = 1.0 / cF  # kD == cF

    def q8(a):
        return np.clip(a, -224.0, 224.0).astype(f8)

    consts = {}
    consts["w12"] = np.concatenate([W12, b12[None, :]], axis=0).astype(bf)
    w34t = np.zeros((128, 2, 512), dtype=np.float64)
    for i in range(2):
        w34t[:, i, :] = kB * W34[128 * i : 128 * (i + 1), :]
    consts["w34"] = q8(w34t.reshape(128, 1024))
    w567t = np.zeros((128, 4, 224), dtype=np.float64)
    for c in range(2):
        for i in range(2):
            blk = kC * W567[256 * c + 128 * i : 256 * c + 128 * (i + 1), :]
            w567t[:, 2 * c + i, 0:100] = blk[:, 0:100]
            w567t[:, 2 * c + i, 112:212] = blk[:, 100:200]
    consts["w567"] = q8(w567t.reshape(128, 896))
    cm2t = np.zeros((100, 2, 80), dtype=np.float64)
    onest = np.zeros((100, 2, 80), dtype=np.float64)
    for i in range(2):
        cm2t[:, i, 0:72] = -2.0 * cp[100 * i : 100 * (i + 1), :]
        onest[:, i, 0:72] = 1.0 / cF
    consts["cm2"] = q8(cm2t.reshape(100, 160))
    consts["onesd"] = q8(onest.reshape(100, 160))
    bbt = np.zeros((128, 4), dtype=np.float64)
    for m in range(4):
        bbt[:, m] = kB * cA * b34[128 * m : 128 * (m + 1)]
    consts["bb"] = bbt.astype(np.float32)
    consts["csq"] = (1.0 + (cp**2).sum(axis=0)).reshape(72, 1).astype(np.float32)

    n_loc = n // N_CORES
    key = (n_loc, cA, sC, sD)
    if key not in _CACHE:
        _CACHE[key] = _build(n_loc, cA, sC, sD)
    nc = _CACHE[key]

    in_maps = []
    for c in range(N_CORES):
        xt = np.empty((73, n_loc), dtype=bf)
        xt[:72] = x[c * n_loc : (c + 1) * n_loc].T
        xt[72] = 1.0
        m = {"xt": np.ascontiguousarray(xt)}
        m.update(consts)
        in_maps.append(m)
    return nc, in_maps


def kernel(
    inputs, w1, b1, w2, b2, w3, b3, w4, b4, w5, b5, w6, b6, w7, b7, center
):
    from concourse.bass_utils import run_bass_kernel_spmd

    inputs_np = {
        "inputs": inputs, "center": center,
        "w1": w1, "b1": b1, "w2": w2, "b2": b2, "w3": w3, "b3": b3,
        "w4": w4, "b4": b4, "w5": w5, "b5": b5, "w6": w6, "b6": b6,
        "w7": w7, "b7": b7,
    }
    nc, in_maps = prepare(inputs_np)
    res = run_bass_kernel_spmd(nc, in_maps, core_ids=list(range(N_CORES)))
    return np.concatenate([res.results[c]["q"] for c in range(N_CORES)], axis=0)
